# revision 1
# baseline (speedup 1.0000x reference)
"""Causal self-attention (GQA + RMS-norm + partial RoPE) Trainium2 kernel.

Full inputs in, full output out. Sharding: 8 cores = batch(4) x head-half(2).
Each core computes its batch's QKV for 8 q-heads / 2 kv-heads entirely in
transposed layouts (head_dim on partitions), does causal attention with a
no-max softmax (scores bounded by sqrt(hd) after RMS norm), and a row-sharded
output projection; the host sums the two half partials per batch.

All large matmuls run in float32r (TF32-like, full PE rate at N=512).
Single long-lived PSUM pool with 8 rotating bank tags (t0..t7) — no
pool-scope barriers anywhere in the hot path.
"""
import numpy as np

import concourse.bacc as bacc
import concourse.mybir as mybir
from concourse.tile import TileContext
from concourse.bass_utils import run_bass_kernel_spmd

F32 = mybir.dt.float32
F32R = mybir.dt.float32r
AF = mybir.ActivationFunctionType

B, S, D = 4, 2048, 2048
H, KV, HD = 16, 4, 128
ROPE, HALF_ROPE = 64, 32
EPS = 1.1920929e-07
N_CORES = 8
NDC = D // 128          # 16 contraction chunks
NQC = S // 512          # 4 query chunks of 512
LH = 8                  # local q heads per core
LKV = 2                 # local kv heads per core

_cached_program = None
_last_in_maps = None


def _build_program():
    nc = bacc.Bacc("TRN2")
    # eps const AP for activation bias
    t = nc.alloc_sbuf_tensor("const-f32-eps", [128, 1], F32)
    nc.gpsimd.memset(t.ap(), EPS)
    nc.const_aps.aps[(F32, EPS)] = t.ap()
    nc.all_engine_barrier()

    xT = nc.declare_dram_parameter("xT", [D, S], F32R, isOutput=False)
    wqT = nc.declare_dram_parameter("wqT", [D, LH * HD], F32R, isOutput=False)
    wkT = nc.declare_dram_parameter("wkT", [D, LKV * HD], F32R, isOutput=False)
    wvT = nc.declare_dram_parameter("wvT", [D, LKV * HD], F32R, isOutput=False)
    wpT = nc.declare_dram_parameter("wpT", [LH * HD, D], F32R, isOutput=False)
    cosT = nc.declare_dram_parameter("cosT", [HALF_ROPE, S], F32, isOutput=False)
    sinT = nc.declare_dram_parameter("sinT", [HALF_ROPE, S], F32, isOutput=False)
    o128d = nc.declare_dram_parameter("ones128", [128, 1], F32R, isOutput=False)
    obd = nc.declare_dram_parameter("onesb", [1, 128], F32R, isOutput=False)
    gaind = nc.declare_dram_parameter("gains", [128, LH], F32, isOutput=False)
    out = nc.declare_dram_parameter("out", [S, D], F32, isOutput=True)

    with TileContext(nc) as tc:
        with (
            tc.tile_pool(name="cp", bufs=1) as cp,
            tc.tile_pool(name="xap", bufs=1) as xap,
            tc.tile_pool(name="wqp", bufs=2) as wqp,
            tc.tile_pool(name="wpp", bufs=2) as wpp,
            tc.tile_pool(name="stgp", bufs=2) as stgp,
            tc.tile_pool(name="scr", bufs=2) as scr,
            tc.tile_pool(name="exp3", bufs=3) as exp3,
            tc.tile_pool(name="qnp", bufs=1) as qnp,
            tc.tile_pool(name="pu", bufs=1, space="PSUM") as pu,
        ):
            wk_t = cp.tile([128, NDC, LKV * HD], F32R, tag="wk")
            nc.sync.dma_start(out=wk_t[:],
                              in_=wkT.rearrange("(c p) e -> p c e", p=128))
            wv_t = cp.tile([128, NDC, LKV * HD], F32R, tag="wv")
            nc.sync.dma_start(out=wv_t[:],
                              in_=wvT.rearrange("(c p) e -> p c e", p=128))
            cos_t = cp.tile([HALF_ROPE, S], F32, tag="cos")
            nc.sync.dma_start(out=cos_t[:], in_=cosT[:])
            sin_t = cp.tile([HALF_ROPE, S], F32, tag="sin")
            nc.sync.dma_start(out=sin_t[:], in_=sinT[:])
            o128 = cp.tile([128, 1], F32R, tag="o128")
            nc.sync.dma_start(out=o128[:], in_=o128d[:])
            ob = cp.tile([1, 128], F32R, tag="ob")
            nc.sync.dma_start(out=ob[:], in_=obd[:])
            gains = cp.tile([128, LH], F32, tag="gains")
            nc.sync.dma_start(out=gains[:], in_=gaind[:])
            kn_t = cp.tile([128, LKV, S], F32R, tag="kn")
            v_t = cp.tile([128, S // 128, LKV * HD], F32R, tag="v")

            def bank(i, shape=(128, 512), dt=F32, nm=None):
                return pu.tile(list(shape), dt, tag=f"t{i}",
                               name=nm or f"pt{i}")

            def load_x(pos0):
                """x chunk [128, NDC, 512] as 16 per-dc DMAs (compute on
                chunk dc can start as soon as its DMA lands)."""
                xa = xap.tile([128, NDC, 512], F32R, tag="xa", name="xa")
                for dc in range(NDC):
                    nc.sync.dma_start(
                        out=xa[:, dc],
                        in_=xT[dc * 128:(dc + 1) * 128, pos0:pos0 + 512])
                return xa

            def norm_rope(raw, out_ap, pos0):
                """RMS-norm + partial RoPE: transposed raw [128,512] (PSUM)
                -> out_ap ([128,512] f32r). pos0 selects cos/sin columns."""
                cs = slice(pos0, pos0 + 512)
                sq = scr.tile([128, 512], F32R, tag="sq", name="sq")
                nc.scalar.activation(sq[:], raw[:], AF.Square)
                ssq = bank(6, (1, 512), nm="ssq")
                nc.tensor.matmul(ssq[:], o128[:], sq[:], start=True, stop=True)
                # r = rsqrt(ssq/128 + eps) = sqrt(1/(ssq/128 + eps))
                u = scr.tile([1, 512], F32, tag="u", name="u")
                nc.scalar.activation(u[:], ssq[:], AF.Copy,
                                     scale=1.0 / HD, bias=EPS)
                uin = scr.tile([1, 512], F32, tag="uin", name="uin")
                nc.vector.reciprocal_approx_fast(out=uin[:], in_=u[:])
                r = scr.tile([1, 512], F32R, tag="rr", name="rr")
                nc.scalar.activation(r[:], uin[:], AF.Sqrt)
                Rp = bank(7, nm="Rp")
                nc.tensor.matmul(Rp[:], ob[:], r[:], start=True, stop=True)
                # cr/sr read R straight from PSUM (1 psum operand is legal);
                # only the rope pass-through rows need an SBUF copy of R
                Rsb = scr.tile([128, 512], F32, tag="Rsb", name="Rsb")
                nc.scalar.copy(Rsb[ROPE:128, :], Rp[ROPE:128, :])
                cr = scr.tile([HALF_ROPE, 512], F32, tag="cr", name="cr")
                sr = scr.tile([HALF_ROPE, 512], F32, tag="sr", name="sr")
                nc.vector.tensor_mul(cr[:], cos_t[:, cs], Rp[0:HALF_ROPE, :])
                nc.vector.tensor_mul(sr[:], sin_t[:, cs], Rp[0:HALF_ROPE, :])
                tmp = scr.tile([ROPE, 512], F32, tag="tmp", name="tmp")
                h1, h2 = slice(0, HALF_ROPE), slice(HALF_ROPE, ROPE)
                nc.vector.tensor_mul(out_ap[h1, :], raw[h1, :], cr[:])
                nc.vector.tensor_mul(tmp[h1, :], raw[h2, :], sr[:])
                nc.vector.tensor_add(out_ap[h1, :], out_ap[h1, :], tmp[h1, :])
                nc.vector.tensor_mul(out_ap[h2, :], raw[h2, :], cr[:])
                nc.vector.tensor_mul(tmp[h2, :], raw[h1, :], sr[:])
                nc.vector.tensor_sub(out_ap[h2, :], out_ap[h2, :], tmp[h2, :])
                nc.vector.tensor_mul(out_ap[ROPE:128, :], raw[ROPE:128, :],
                                     Rsb[ROPE:128, :])

            # ---------------- Phase A: kT (norm+rope) and v ----------------
            for sc in range(NQC):
                xa = load_x(sc * 512)
                for g in range(LKV):
                    kacc = bank(g, nm=f"kacc{g}")
                    for dc in range(NDC):
                        nc.tensor.matmul(
                            kacc[:], wk_t[:, dc, g * HD:(g + 1) * HD],
                            xa[:, dc], start=(dc == 0), stop=(dc == NDC - 1))
                    norm_rope(kacc, kn_t[:, g, sc * 512:(sc + 1) * 512],
                              sc * 512)
                for st4 in range(4):
                    st = sc * 4 + st4
                    vacc = bank(2 + st4 % 2, (128, LKV * HD), nm=f"vacc{st4}")
                    for dc in range(NDC):
                        nc.tensor.matmul(
                            vacc[:], xa[:, dc, st4 * 128:(st4 + 1) * 128],
                            wv_t[:, dc], start=(dc == 0), stop=(dc == NDC - 1))
                    nc.vector.tensor_copy(v_t[:, st], vacc[:])

            # ------------- Phase C: per query chunk q/attn/proj -------------
            for qc in range(NQC):
                pos0 = qc * 512
                n_kt = (qc + 1) * 4

                # -- q projection (two groups of 4 heads) + norm + rope --
                xa = load_x(pos0)
                qn = {}
                for grp in range(2):
                    qraw = {}
                    for dc in range(NDC):
                        wqt = wqp.tile([128, 512], F32R, tag="wq", name="wq")
                        nc.sync.dma_start(
                            out=wqt[:],
                            in_=wqT[dc * 128:(dc + 1) * 128,
                                    grp * 512:(grp + 1) * 512])
                        for hh in range(4):
                            if dc == 0:
                                qraw[hh] = bank(hh, nm=f"qraw{hh}")
                            nc.tensor.matmul(
                                qraw[hh][:], wqt[:, hh * HD:(hh + 1) * HD],
                                xa[:, dc], start=(dc == 0),
                                stop=(dc == NDC - 1))
                    for hh in range(4):
                        h = grp * 4 + hh
                        qn[h] = qnp.tile([128, 512], F32R, tag=f"qn{h}",
                                         name=f"qn{h}")
                        norm_rope(qraw[hh], qn[h][:], pos0)

                # -- attention --
                yt_sb = {}
                for h in range(LH):
                    g = h // 4
                    yt_ps = bank(h % 2, nm=f"yt{h}")
                    l_ps = bank(2 + h % 2, (1, 512), nm=f"l{h}")
                    for kt in range(n_kt):
                        sc_ps = bank(4 + kt % 4, nm=f"sc{h}_{kt}")
                        nc.tensor.matmul(
                            sc_ps[:], kn_t[:, g, kt * 128:(kt + 1) * 128],
                            qn[h][:], start=True, stop=True)
                        ex = exp3.tile([128, 512], F32R, tag="ex", name="ex")
                        nc.scalar.activation(ex[:], sc_ps[:], AF.Exp,
                                             scale=gains[:, h:h + 1])
                        m = kt - qc * 4
                        if m >= 0:
                            # zero ex where key pos > query pos:
                            # keep iff n - p - 128*m >= 0
                            nc.gpsimd.affine_select(
                                out=ex[:], in_=ex[:],
                                compare_op=mybir.AluOpType.is_ge,
                                fill=0.0, base=-128 * m,
                                pattern=[[1, 512]], channel_multiplier=-1)
                        nc.tensor.matmul(
                            yt_ps[:], v_t[:, kt, g * HD:(g + 1) * HD], ex[:],
                            start=(kt == 0), stop=(kt == n_kt - 1))
                        nc.tensor.matmul(
                            l_ps[:], o128[:], ex[:], start=(kt == 0),
                            stop=(kt == n_kt - 1))
                    lf = scr.tile([1, 512], F32, tag="lf", name="lf")
                    nc.vector.reciprocal_approx_fast(out=lf[:], in_=l_ps[:])
                    linv = scr.tile([1, 512], F32R, tag="linv", name="linv")
                    nc.scalar.copy(linv[:], lf[:])
                    Li_ps = bank(4 + (n_kt + h) % 4, nm=f"Li{h}")
                    nc.tensor.matmul(Li_ps[:], ob[:], linv[:],
                                     start=True, stop=True)
                    Lsb = scr.tile([128, 512], F32, tag="Lsb", name="Lsb")
                    nc.vector.tensor_copy(Lsb[:], Li_ps[:])
                    yt_sb[h] = qnp.tile([128, 512], F32R, tag=f"yts{h}",
                                        name=f"yts{h}")
                    nc.vector.tensor_mul(yt_sb[h][:], yt_ps[:], Lsb[:])

                # -- output projection: out[s_q, j] += yT.T @ wpT --
                # 4 j-columns of 512; 4 psum accumulators (t4..t7) per jcol
                for jcol in range(4):
                    prs = {}
                    for h in range(LH):
                        wpt = wpp.tile([128, 512], F32R, tag="wp", name="wp")
                        nc.sync.dma_start(
                            out=wpt[:],
                            in_=wpT[h * 128:(h + 1) * 128,
                                    jcol * 512:(jcol + 1) * 512])
                        for st4 in range(4):
                            if h == 0:
                                prs[st4] = bank(4 + st4, nm=f"pr{jcol}{st4}")
                            nc.tensor.matmul(
                                prs[st4][:],
                                yt_sb[h][:, st4 * 128:(st4 + 1) * 128],
                                wpt[:], start=(h == 0), stop=(h == LH - 1))
                    for st4 in range(4):
                        stg = stgp.tile([128, 512], F32, tag="stg", name="stg")
                        nc.vector.tensor_copy(stg[:], prs[st4][:])
                        nc.sync.dma_start(
                            out=out[pos0 + st4 * 128:pos0 + (st4 + 1) * 128,
                                    jcol * 512:(jcol + 1) * 512],
                            in_=stg[:])
    nc.compile()
    return nc


def _rope_tables():
    inv = 1.0 / (10000.0 ** (np.arange(0, ROPE, 2, dtype=np.float64) / ROPE))
    fr = np.outer(np.arange(S, dtype=np.float64), inv)  # [S, 32]
    return (np.cos(fr).T.astype(np.float32).copy(),
            np.sin(fr).T.astype(np.float32).copy())


def kernel(x, Wq, Wk, Wv, Wproj, q_gain):
    global _cached_program, _last_in_maps
    x = np.ascontiguousarray(np.asarray(x, dtype=np.float32))
    Wq = np.asarray(Wq, dtype=np.float32)
    Wk = np.asarray(Wk, dtype=np.float32)
    Wv = np.asarray(Wv, dtype=np.float32)
    Wproj = np.asarray(Wproj, dtype=np.float32)
    q_gain = np.asarray(q_gain, dtype=np.float32)

    cosT, sinT = _rope_tables()
    ones128 = np.ones((128, 1), dtype=np.float32)
    onesb = np.ones((1, 128), dtype=np.float32)
    scale = 1.0 / np.sqrt(HD)

    in_maps = []
    for core in range(N_CORES):
        b, half = core // 2, core % 2
        g0 = half * LKV
        gains = np.repeat((q_gain[half * LH:(half + 1) * LH] * scale)
                          [None, :], 128, axis=0).astype(np.float32)
        in_maps.append({
            "xT": np.ascontiguousarray(x[b].T),
            "wqT": np.ascontiguousarray(
                Wq[half * LH * HD:(half + 1) * LH * HD, :].T),
            "wkT": np.ascontiguousarray(
                Wk[g0 * HD:(g0 + LKV) * HD, :].T),
            "wvT": np.ascontiguousarray(
                Wv[g0 * HD:(g0 + LKV) * HD, :].T),
            "wpT": np.ascontiguousarray(
                Wproj[:, half * LH * HD:(half + 1) * LH * HD].T),
            "cosT": cosT, "sinT": sinT,
            "ones128": ones128, "onesb": onesb, "gains": gains,
        })

    _last_in_maps = in_maps
    if _cached_program is None:
        _cached_program = _build_program()
    res = run_bass_kernel_spmd(_cached_program, in_maps, list(range(N_CORES)))

    out = np.empty((B, S, D), dtype=np.float32)
    for b in range(B):
        out[b] = res.results[2 * b]["out"] + res.results[2 * b + 1]["out"]
    return out



# revision 7
# speedup vs baseline: 1.8512x; 1.8512x over previous
"""Causal self-attention (GQA + RMS-norm + partial RoPE) Trainium2 kernel, v2.

Full inputs in, full output out. Sharding: 8 cores = batch(4) x head-half(2).
Each core computes QKV for 8 q-heads / 2 kv-heads of one batch, causal
attention with a no-max softmax, and a row-sharded output projection; the host
sums the two half partials per batch.

v2 vs v1: all-bf16 datapath (fp32r measured ~2 cyc/row on HW vs bf16 1),
all weights SBUF-resident (loaded once), x streamed once (q projection moved
into Phase A), exp batched over [128,1024] two-bank PSUM score tiles, paired
heads sharing kn/v stationaries, sqrt-set ACT ops confined to Phase A so the
exp table set loads once for Phase C.
"""
import numpy as np
import ml_dtypes

import concourse.bacc as bacc
import concourse.mybir as mybir
from concourse.tile import TileContext
from concourse.bass_utils import run_bass_kernel_spmd

F32 = mybir.dt.float32
BF = mybir.dt.bfloat16
AF = mybir.ActivationFunctionType
NPBF = ml_dtypes.bfloat16

B, S, D = 4, 2048, 2048
H, KV, HD = 16, 4, 128
ROPE, HALF_ROPE = 64, 32
EPS = 1.1920929e-07
N_CORES = 8
NDC = D // 128          # 16 contraction chunks
NQC = S // 512          # 4 query chunks of 512
LH = 8                  # local q heads per core
LKV = 2                 # local kv heads per core

_cached_program = None
_last_in_maps = None


def _build_program():
    nc = bacc.Bacc("TRN2")
    t = nc.alloc_sbuf_tensor("const-f32-eps", [128, 1], F32)
    nc.gpsimd.memset(t.ap(), EPS)
    nc.const_aps.aps[(F32, EPS)] = t.ap()
    nc.all_engine_barrier()

    xT = nc.declare_dram_parameter("xT", [D, S], BF, isOutput=False)
    wqT = nc.declare_dram_parameter("wqT", [D, LH * HD], BF, isOutput=False)
    wkT = nc.declare_dram_parameter("wkT", [D, LKV * HD], BF, isOutput=False)
    wvT = nc.declare_dram_parameter("wvT", [D, LKV * HD], BF, isOutput=False)
    wpT = nc.declare_dram_parameter("wpT", [LH * HD, D], BF, isOutput=False)
    # cos/sin tables duplicated across both 32-row halves so rope tensor ops
    # always pair SBUF inputs with equal base partitions (walrus NCC_IBIR297)
    cosT = nc.declare_dram_parameter("cosT", [ROPE, S], BF, isOutput=False)
    sinT = nc.declare_dram_parameter("sinT", [ROPE, S], BF, isOutput=False)
    o128d = nc.declare_dram_parameter("ones128", [128, 1], BF, isOutput=False)
    obd = nc.declare_dram_parameter("onesb", [1, 128], BF, isOutput=False)
    gaind = nc.declare_dram_parameter("gains", [128, LH], F32, isOutput=False)
    out = nc.declare_dram_parameter("out", [S, D], F32, isOutput=True)

    with TileContext(nc) as tc:
        with (
            tc.tile_pool(name="cw", bufs=1) as cw,
            tc.tile_pool(name="xap", bufs=1) as xap,
            tc.tile_pool(name="scr", bufs=2) as scr,
            tc.tile_pool(name="exq", bufs=3) as exq,
            tc.tile_pool(name="ytp", bufs=1) as ytp,
            tc.tile_pool(name="stgp", bufs=2) as stgp,
            tc.tile_pool(name="pub", bufs=2, space="PSUM") as pub,
            tc.tile_pool(name="pus", bufs=4, space="PSUM") as pus,
        ):
            # ---- resident weights / tables (loaded once) ----
            wq_t = cw.tile([128, NDC, LH * HD], BF, tag="wq")
            nc.sync.dma_start(out=wq_t[:],
                              in_=wqT.rearrange("(c p) e -> p c e", p=128))
            wk_t = cw.tile([128, NDC, LKV * HD], BF, tag="wk")
            nc.sync.dma_start(out=wk_t[:],
                              in_=wkT.rearrange("(c p) e -> p c e", p=128))
            wv_t = cw.tile([128, NDC, LKV * HD], BF, tag="wv")
            nc.sync.dma_start(out=wv_t[:],
                              in_=wvT.rearrange("(c p) e -> p c e", p=128))
            wp_t = cw.tile([128, LH, D], BF, tag="wp")
            nc.sync.dma_start(out=wp_t[:],
                              in_=wpT.rearrange("(c p) e -> p c e", p=128))
            cos_t = cw.tile([ROPE, S], BF, tag="cos")
            nc.sync.dma_start(out=cos_t[:], in_=cosT[:])
            sin_t = cw.tile([ROPE, S], BF, tag="sin")
            nc.sync.dma_start(out=sin_t[:], in_=sinT[:])
            o128 = cw.tile([128, 1], BF, tag="o128")
            nc.sync.dma_start(out=o128[:], in_=o128d[:])
            ob = cw.tile([1, 128], BF, tag="ob")
            nc.sync.dma_start(out=ob[:], in_=obd[:])
            gains = cw.tile([128, LH], F32, tag="gains")
            nc.sync.dma_start(out=gains[:], in_=gaind[:])
            # ---- resident activations ----
            kn_t = cw.tile([128, LKV, S], BF, tag="kn")
            v_t = cw.tile([128, S // 128, LKV * HD], BF, tag="v")
            qn_t = cw.tile([128, LH, S], BF, tag="qn")

            def norm_rope(raw, out_ap, pos0):
                """RMS-norm + partial RoPE: raw [128,512] f32 PSUM slice ->
                out_ap [128,512] bf16. pos0 selects cos/sin columns."""
                cs = slice(pos0, pos0 + 512)
                rawb = scr.tile([128, 512], BF, tag="rawb", name="rawb")
                nc.scalar.copy(rawb[:], raw[:])
                sq = scr.tile([128, 512], BF, tag="sq", name="sq")
                nc.scalar.activation(sq[:], rawb[:], AF.Square)
                ssq = pus.tile([1, 512], F32, tag="sm", name="ssq")
                nc.tensor.matmul(ssq[:], o128[:], sq[:], start=True, stop=True)
                u = scr.tile([1, 512], F32, tag="u", name="u")
                nc.scalar.activation(u[:], ssq[:], AF.Copy,
                                     scale=1.0 / HD, bias=EPS)
                uin = scr.tile([1, 512], F32, tag="uin", name="uin")
                nc.vector.reciprocal_approx_fast(out=uin[:], in_=u[:])
                r = scr.tile([1, 512], BF, tag="rr", name="rr")
                nc.scalar.activation(r[:], uin[:], AF.Sqrt)
                Rp = pus.tile([128, 512], F32, tag="sm", name="Rp")
                nc.tensor.matmul(Rp[:], ob[:], r[:], start=True, stop=True)
                Rb = scr.tile([128, 512], BF, tag="Rb", name="Rb")
                nc.scalar.copy(Rb[:], Rp[:])
                # cr/sr hold cos*R (sin*R) duplicated in rows 0:32 and 32:64
                cr = scr.tile([ROPE, 512], BF, tag="cr", name="cr")
                sr = scr.tile([ROPE, 512], BF, tag="sr", name="sr")
                nc.vector.tensor_mul(cr[:], cos_t[:, cs], Rb[0:ROPE, :])
                nc.vector.tensor_mul(sr[:], sin_t[:, cs], Rb[0:ROPE, :])
                tmp = scr.tile([ROPE, 512], BF, tag="tmp", name="tmp")
                h1, h2 = slice(0, HALF_ROPE), slice(HALF_ROPE, ROPE)
                nc.vector.tensor_mul(out_ap[h1, :], rawb[h1, :], cr[h1, :])
                nc.vector.tensor_mul(tmp[h1, :], rawb[h2, :], sr[h2, :])
                nc.vector.tensor_add(out_ap[h1, :], out_ap[h1, :], tmp[h1, :])
                nc.vector.tensor_mul(out_ap[h2, :], rawb[h2, :], cr[h2, :])
                nc.vector.tensor_mul(tmp[h2, :], rawb[h1, :], sr[h1, :])
                nc.vector.tensor_sub(out_ap[h2, :], out_ap[h2, :], tmp[h2, :])
                nc.vector.tensor_mul(out_ap[ROPE:128, :], rawb[ROPE:128, :],
                                     Rb[ROPE:128, :])

            # ================= Phase A: k, v, q for all positions ==========
            for sc in range(NQC):
                pos0 = sc * 512
                xa = xap.tile([128, NDC, 512], BF, tag="xa", name="xa")
                for dc in range(NDC):
                    nc.sync.dma_start(
                        out=xa[:, dc],
                        in_=xT[dc * 128:(dc + 1) * 128, pos0:pos0 + 512])
                # k projection + norm/rope
                for g in range(LKV):
                    kacc = pus.tile([128, 512], F32, tag="sm", name=f"kacc{g}")
                    for dc in range(NDC):
                        nc.tensor.matmul(
                            kacc[:], wk_t[:, dc, g * HD:(g + 1) * HD],
                            xa[:, dc], start=(dc == 0), stop=(dc == NDC - 1))
                    norm_rope(kacc, kn_t[:, g, pos0:pos0 + 512], pos0)
                # v projection
                for st4 in range(4):
                    vacc = pus.tile([128, LKV * HD], F32, tag="sm",
                                   name=f"vacc{st4}")
                    for dc in range(NDC):
                        nc.tensor.matmul(
                            vacc[:], xa[:, dc, st4 * 128:(st4 + 1) * 128],
                            wv_t[:, dc], start=(dc == 0), stop=(dc == NDC - 1))
                    nc.vector.tensor_copy(v_t[:, sc * 4 + st4], vacc[:])
                # q projection (two big tiles of 2 heads each per grp)
                for grp in range(2):
                    big0 = pub.tile([128, 1024], F32, tag="big",
                                   name=f"qr{grp}0")
                    big1 = pub.tile([128, 1024], F32, tag="big",
                                   name=f"qr{grp}1")
                    for dc in range(NDC):
                        for hh in range(4):
                            h = grp * 4 + hh
                            tgt = (big0 if hh < 2 else big1)
                            half = (hh % 2) * 512
                            nc.tensor.matmul(
                                tgt[:, half:half + 512],
                                wq_t[:, dc, h * HD:(h + 1) * HD],
                                xa[:, dc], start=(dc == 0),
                                stop=(dc == NDC - 1))
                    for hh in range(4):
                        h = grp * 4 + hh
                        tgt = (big0 if hh < 2 else big1)
                        half = (hh % 2) * 512
                        norm_rope(tgt[:, half:half + 512],
                                  qn_t[:, h, pos0:pos0 + 512], pos0)

            # ================= Phase C: attention + output proj =============
            for qc in range(NQC):
                pos0 = qc * 512
                qcs = slice(pos0, pos0 + 512)
                n_kt = (qc + 1) * 4
                n_grp = n_kt // 2
                ytn = {}
                for h0 in range(0, LH, 2):
                    h1 = h0 + 1
                    g = h0 // 4
                    yt = {h0: pus.tile([128, 512], F32, tag="sm",
                                      name=f"yt{h0}"),
                          h1: pus.tile([128, 512], F32, tag="sm",
                                      name=f"yt{h1}")}
                    lt = {h0: pus.tile([1, 512], F32, tag="sm",
                                      name=f"l{h0}"),
                          h1: pus.tile([1, 512], F32, tag="sm",
                                      name=f"l{h1}")}
                    for j in range(n_grp):
                        for h in (h0, h1):
                            sc2 = pub.tile([128, 1024], F32, tag="big",
                                          name=f"sc{h}_{j}")
                            for m in range(2):
                                kt = 2 * j + m
                                nc.tensor.matmul(
                                    sc2[:, m * 512:(m + 1) * 512],
                                    kn_t[:, g, kt * 128:(kt + 1) * 128],
                                    qn_t[:, h, qcs], start=True, stop=True)
                            ex = exq.tile([128, 1024], BF, tag="ex",
                                          name="ex")
                            nc.scalar.activation(ex[:], sc2[:], AF.Exp,
                                                 scale=gains[:, h:h + 1])
                            if 2 * j + 1 >= 4 * qc:
                                # group overlaps the causal diagonal:
                                # keep iff 512*qc + n - 128*(2j+m) - p >= 0
                                exv = ex[:].rearrange("p (m n) -> p m n", m=2)
                                nc.gpsimd.affine_select(
                                    out=exv, in_=exv,
                                    compare_op=mybir.AluOpType.is_ge,
                                    fill=0.0, base=512 * qc - 256 * j,
                                    pattern=[[-128, 2], [1, 512]],
                                    channel_multiplier=-1)
                            for m in range(2):
                                kt = 2 * j + m
                                nc.tensor.matmul(
                                    yt[h][:],
                                    v_t[:, kt, g * HD:(g + 1) * HD],
                                    ex[:, m * 512:(m + 1) * 512],
                                    start=(kt == 0), stop=(kt == n_kt - 1))
                                nc.tensor.matmul(
                                    lt[h][:], o128[:],
                                    ex[:, m * 512:(m + 1) * 512],
                                    start=(kt == 0), stop=(kt == n_kt - 1))
                    # finalize pair: y = yt / l
                    for h in (h0, h1):
                        lf = scr.tile([1, 512], F32, tag="lf", name="lf")
                        nc.vector.reciprocal_approx_fast(out=lf[:],
                                                         in_=lt[h][:])
                        linv = scr.tile([1, 512], BF, tag="linv", name="linv")
                        nc.vector.tensor_copy(linv[:], lf[:])
                        Li = pus.tile([128, 512], F32, tag="sm", name=f"Li{h}")
                        nc.tensor.matmul(Li[:], ob[:], linv[:],
                                         start=True, stop=True)
                        Lsb = scr.tile([128, 512], BF, tag="Lsb", name="Lsb")
                        nc.vector.tensor_copy(Lsb[:], Li[:])
                        ytn[h] = ytp.tile([128, 512], BF, tag=f"yts{h}",
                                          name=f"yts{h}")
                        nc.vector.tensor_mul(ytn[h][:], yt[h][:], Lsb[:])

                # -- output projection: out[s_q, j] += ytn.T @ wp --
                for jc in range(4):
                    jcs = slice(jc * 512, (jc + 1) * 512)
                    for st4 in range(4):
                        pr = pus.tile([128, 512], F32, tag="sm",
                                     name=f"pr{jc}{st4}")
                        for h in range(LH):
                            nc.tensor.matmul(
                                pr[:],
                                ytn[h][:, st4 * 128:(st4 + 1) * 128],
                                wp_t[:, h, jcs], start=(h == 0),
                                stop=(h == LH - 1))
                        stg = stgp.tile([128, 512], F32, tag="stg",
                                        name="stg")
                        nc.vector.tensor_copy(stg[:], pr[:])
                        nc.sync.dma_start(
                            out=out[pos0 + st4 * 128:pos0 + (st4 + 1) * 128,
                                    jcs],
                            in_=stg[:])
    nc.compile()
    return nc


def _rope_tables():
    inv = 1.0 / (10000.0 ** (np.arange(0, ROPE, 2, dtype=np.float64) / ROPE))
    fr = np.outer(np.arange(S, dtype=np.float64), inv)  # [S, 32]
    c = np.cos(fr).T.astype(NPBF)
    s = np.sin(fr).T.astype(NPBF)
    # duplicate rows 0:32 into 32:64 (see kernel comment on NCC_IBIR297)
    return (np.concatenate([c, c], axis=0).copy(),
            np.concatenate([s, s], axis=0).copy())


def kernel(x, Wq, Wk, Wv, Wproj, q_gain):
    global _cached_program, _last_in_maps
    x = np.asarray(x, dtype=np.float32)
    Wq = np.asarray(Wq, dtype=np.float32)
    Wk = np.asarray(Wk, dtype=np.float32)
    Wv = np.asarray(Wv, dtype=np.float32)
    Wproj = np.asarray(Wproj, dtype=np.float32)
    q_gain = np.asarray(q_gain, dtype=np.float32)

    cosT, sinT = _rope_tables()
    ones128 = np.ones((128, 1), dtype=NPBF)
    onesb = np.ones((1, 128), dtype=NPBF)
    scale = 1.0 / np.sqrt(HD)

    in_maps = []
    for core in range(N_CORES):
        b, half = core // 2, core % 2
        g0 = half * LKV
        gains = np.repeat((q_gain[half * LH:(half + 1) * LH] * scale)
                          [None, :], 128, axis=0).astype(np.float32)
        in_maps.append({
            "xT": np.ascontiguousarray(x[b].T).astype(NPBF),
            "wqT": np.ascontiguousarray(
                Wq[half * LH * HD:(half + 1) * LH * HD, :].T).astype(NPBF),
            "wkT": np.ascontiguousarray(
                Wk[g0 * HD:(g0 + LKV) * HD, :].T).astype(NPBF),
            "wvT": np.ascontiguousarray(
                Wv[g0 * HD:(g0 + LKV) * HD, :].T).astype(NPBF),
            "wpT": np.ascontiguousarray(
                Wproj[:, half * LH * HD:(half + 1) * LH * HD].T).astype(NPBF),
            "cosT": cosT, "sinT": sinT,
            "ones128": ones128, "onesb": onesb, "gains": gains,
        })

    _last_in_maps = in_maps
    if _cached_program is None:
        _cached_program = _build_program()
    res = run_bass_kernel_spmd(_cached_program, in_maps, list(range(N_CORES)))

    out = np.empty((B, S, D), dtype=np.float32)
    for b in range(B):
        out[b] = res.results[2 * b]["out"] + res.results[2 * b + 1]["out"]
    return out
